# revision 11
# baseline (speedup 1.0000x reference)
"""Trainium2 Bass kernel for nn_DWTExtractor.

Computes, for each single-channel 1024x1024 image, 6 output channels
(3 Haar DWT2 details + 3 Coif1 DWT2 details bilinearly resized to 512x512).

Everything is linear and separable, so each channel is
    chan = RowM @ img @ ColM^T
with RowM/ColM in {Hlo, Hhi, RClo, RChi} (all [512, 1024] banded operators;
RC* fold the coif1 DWT with the jax.image.resize 514->512 linear+antialias
matrix). Both passes run on the TensorEngine with the *data* as the
stationary operand (lhsT), so each pass's output lands in PSUM already
transposed for the next pass - no transpose instructions at all:

  pass 1: T2[op][w, n] = sum_h X[h, w] * Op[n, h]
          lhsT = X[h-window, w-window] (128x128), rhs = packed band matrix
  pass 2: chan[m, n]   = sum_w T2[op][w, m] * Col[n, w]
          lhsT = T2[w-window, m-slice], rhs = band slice

The image axes are covered by 9 overlapping 128-wide windows (stride 114)
so that every output column's 12-tap support lies inside a single window;
each window writes a disjoint column slice (singleton PSUM groups, no
cross-window accumulation). Pass-1 packs all 4 operators' bands into one
[128, 228] rhs per window -> one matmul per (w-window, h-window).

v2 over the baseline:
  - output in fp16 (host upcasts): halves the dominant HBM write traffic
  - input pre-windowed on host -> 1 input DMA per image, 1 weight DMA,
    2 output DMAs per image (13 dma_start total vs 141; each trigger
    costs ~630ns serialized on the Sync engine / HWDGE)
  - PSUM evacuation split across Vector+Scalar+GpSimd (3 engines)
  - pass-1 (image i+1) chunks interleaved with pass-2 (image i) chunks
    in the in-order PE queue so PSUM-reuse waits never stall the PE

Sharding: pure data parallel, 32 images -> 8 cores x 4 images.
"""

import sys

sys.path.insert(0, "/opt/trn_rl_repo")

from contextlib import ExitStack

import numpy as np

import concourse.bass as bass
from concourse import bacc
import concourse.mybir as mybir
import concourse.tile as tile
from concourse.bass_utils import run_bass_kernel_spmd

# ---------------------------------------------------------------------------
# Host-side operator construction (pure numpy, float64)
# ---------------------------------------------------------------------------

_c = np.array([-0.01565572813546454, -0.0727326195128539, 0.38486484686420286,
               0.8525720202122554, 0.3378976624578092, -0.0727326195128539])
HAAR_LO = np.array([0.7071067811865476, 0.7071067811865476])
HAAR_HI = np.array([-0.7071067811865476, 0.7071067811865476])
COIF1_LO = _c.copy()
COIF1_HI = ((-1.0) ** (np.arange(6) + 1)) * _c[::-1]

H = 1024
NT = 9            # overlapping 128-row windows, stride 114
SLOT = 57         # output columns assigned per window (57 * 9 = 513)
ROW_START = [min(max(114 * t - 6, 0), H - 128) for t in range(NT)]
N_CORES = 8
B_TOTAL = 32
BPC = B_TOTAL // N_CORES

# channel -> (row op index, col op index); ops are [Hlo, Hhi, RClo, RChi]
CHAN_OPS = [(1, 0), (0, 1), (1, 1), (3, 2), (2, 3), (3, 3)]

DT = mybir.dt.float16
NPDT = np.float16
F32 = mybir.dt.float32


def _dwt1d_np(x, filt):
    L = len(filt)
    n = x.shape[-1]
    xp = np.pad(x, [(0, 0)] * (x.ndim - 1) + [(L - 1, L - 1)], mode="symmetric")
    out_len = (n + L - 1) // 2
    fr = filt[::-1]
    y = np.zeros(x.shape[:-1] + (out_len,), dtype=x.dtype)
    for j in range(L):
        y = y + fr[j] * xp[..., 1 + j:1 + j + 2 * out_len:2]
    return y


def _dwt_matrix(n, filt):
    eye = np.eye(n, dtype=np.float64)
    return _dwt1d_np(eye, np.asarray(filt, np.float64)).T.copy()


def _resize_matrix(in_size, out_size):
    """Replicates jax.image.resize(method='linear', antialias=True)."""
    scale = out_size / in_size
    inv_scale = 1.0 / scale
    kernel_scale = max(inv_scale, 1.0)
    sample_f = (np.arange(out_size, dtype=np.float64) + 0.5) * inv_scale - 0.5
    x = np.abs(sample_f[None, :]
               - np.arange(in_size, dtype=np.float64)[:, None]) / kernel_scale
    w = np.maximum(0.0, 1.0 - x)
    total = w.sum(axis=0, keepdims=True)
    w = np.where(np.abs(total) > 1000.0 * np.finfo(np.float32).eps,
                 w / np.where(total != 0, total, 1), 0.0)
    w = np.where(((sample_f >= -0.5) & (sample_f <= in_size - 0.5))[None, :],
                 w, 0.0)
    return w.T.copy()


def build_ops():
    Hlo = _dwt_matrix(H, HAAR_LO)
    Hhi = _dwt_matrix(H, HAAR_HI)
    Clo = _dwt_matrix(H, COIF1_LO)
    Chi = _dwt_matrix(H, COIF1_HI)
    R = _resize_matrix(514, 512)
    return [Hlo, Hhi, R @ Clo, R @ Chi]


def assigned(t):
    return SLOT * t, min(SLOT * (t + 1), 512)


def build_bands(ops):
    """wmat [NT, 128, 4*SLOT]: per-window packed band matrices."""
    wmat = np.zeros((NT, 128, 4 * SLOT), np.float64)
    for t in range(NT):
        rs = ROW_START[t]
        n0, n1 = assigned(t)
        for f in range(4):
            full = ops[f][n0:n1]
            mask = np.zeros(H, bool)
            mask[rs:rs + 128] = True
            assert np.abs(full[:, ~mask]).max() == 0.0, (t, f)
            wmat[t, :, f * SLOT:f * SLOT + (n1 - n0)] = full[:, rs:rs + 128].T
    return wmat


# ---------------------------------------------------------------------------
# Bass kernel
# ---------------------------------------------------------------------------

def build_nc(bpc=BPC):
    nc = bacc.Bacc("TRN2", num_swdge_queues=4)
    # x pre-windowed on host: x[i, p, t, w] = img[i, ROW_START[t]+p, w]
    x = nc.dram_tensor("x", [bpc, 128, NT, H], DT, kind="ExternalInput")
    w = nc.dram_tensor("w", [128, NT, 4 * SLOT], DT, kind="ExternalInput")
    # y[i, p, c, rb, n] = chan[c][128*rb + p, n]
    y = nc.dram_tensor("y", [bpc, 128, 6, 4, 512], DT, kind="ExternalOutput")

    with tile.TileContext(nc) as tc, ExitStack() as ctx:
        const = ctx.enter_context(tc.tile_pool(name="const", bufs=1))
        xpool = ctx.enter_context(tc.tile_pool(name="xpool", bufs=1))
        t2p = ctx.enter_context(tc.tile_pool(name="t2p", bufs=3))
        outs = ctx.enter_context(tc.tile_pool(name="outs", bufs=2))
        psum = ctx.enter_context(tc.tile_pool(name="psum", bufs=1,
                                              space="PSUM"))

        xtiles = {}

        def load_x(i):
            xt = xpool.tile([128, NT, H], DT, name=f"x{i}", tag=f"x{i % 2}")
            nc.sync.dma_start(xt[:], x[i])
            xtiles[i] = xt

        # weights first (tiny; pass-1 needs them before anything runs),
        # then the first images' big transfers.
        wt = const.tile([128, NT, 4 * SLOT], DT, name="w", tag="w")
        nc.sync.dma_start(wt[:], w[:])
        for i in range(min(2, bpc)):
            load_x(i)

        def eng_copy(k, dst, src):
            # GPSIMD cannot access PSUM; only Vector/Scalar can evacuate.
            if k == 0:
                nc.vector.tensor_copy(dst, src)
            else:
                nc.scalar.copy(dst, src)

        def p1_chunks(i, t2s):
            """9 chunks: per w-window, 9 matmuls + 2-engine evacuation.

            PSUM is split A/B (slots 0-4 / 5-8) so the copies of one half
            overlap the matmuls of the other half; t2 is split by operator
            pair (ops 0,1 -> t2a written only by Vector, ops 2,3 -> t2b
            written only by Scalar) so the two evacuation copies never
            serialize on a shared destination tile.
            """
            def chunk(wt_i):
                ws = ROW_START[wt_i]
                t2a = t2p.tile([128, 2 * 513], DT,
                               name=f"t2a_{wt_i}", tag=f"t2a_{wt_i}")
                t2b = t2p.tile([128, 2 * 513], DT,
                               name=f"t2b_{wt_i}", tag=f"t2b_{wt_i}")
                t2ar = t2a.rearrange("p (f s j) -> p s f j",
                                     f=2, s=NT, j=SLOT)
                t2br = t2b.rearrange("p (f s j) -> p s f j",
                                     f=2, s=NT, j=SLOT)
                for half in range(2):
                    s0, s1 = (0, 5) if half == 0 else (5, NT)
                    pt = psum.tile([128, 256 * (s1 - s0)], F32,
                                   name=f"pt{half}", tag=f"pt{half}")
                    for ht in range(s0, s1):
                        nc.tensor.matmul(
                            pt[:, 256 * (ht - s0):256 * (ht - s0) + 4 * SLOT],
                            lhsT=xtiles[i][:, ht, ws:ws + 128],
                            rhs=wt[:, ht, :],
                            start=True, stop=True)
                    src = pt.rearrange("p (s c) -> p s c", c=256)[
                        :, :, 0:4 * SLOT].rearrange(
                        "p s (f j) -> p s f j", j=SLOT)
                    nc.vector.tensor_copy(t2ar[:, s0:s1], src[:, :, 0:2])
                    nc.scalar.copy(t2br[:, s0:s1], src[:, :, 2:4])
                t2s[wt_i] = (t2a, t2b)
            return [lambda wt_i=wt_i: chunk(wt_i) for wt_i in range(NT)]

        def p2_chunks(i, t2s):
            """16 matmul/copy chunks + 2 output-DMA chunks for image i."""
            ot01 = outs.tile([128, 6, 2, 512], DT, name="o01", tag="o01")
            ot23 = outs.tile([128, 6, 2, 512], DT, name="o23", tag="o23")
            rr = [0]
            chunks = []

            def chunk(rb, group, ot):
                ptc = {}
                for c in group:
                    ptc[c] = psum.tile([128, 512], F32,
                                       name=f"pc{c}", tag="pc", bufs=3)
                for wt_i in range(NT):
                    n0, n1 = assigned(wt_i)
                    for c in group:
                        ri, ci = CHAN_OPS[c]
                        t2t = t2s[wt_i][ri // 2]
                        nc.tensor.matmul(
                            ptc[c][:, n0:n1],
                            lhsT=t2t[:, 513 * (ri % 2) + 128 * rb:
                                     513 * (ri % 2) + 128 * rb + 128],
                            rhs=wt[:, wt_i, SLOT * ci:SLOT * ci + (n1 - n0)],
                            start=True, stop=True)
                for c in group:
                    # 11-of-24 to Vector, 13 to Scalar: with the 1:1 pass-1
                    # split this balances DVE 0.96GHz vs Act 1.2GHz + the
                    # larger per-op access overhead on Act.
                    eng_copy(0 if (rr[0] % 2 == 0 and rr[0] % 24 != 22) else 1,
                             ot[:, c, rb % 2, :], ptc[c][:])
                    rr[0] += 1

            for rb in range(4):
                ot = ot01 if rb < 2 else ot23
                for group in ((0, 2), (1,), (3, 5), (4,)):
                    chunks.append(
                        lambda rb=rb, group=group, ot=ot: chunk(rb, group, ot))
                if rb == 1:
                    chunks.append(lambda: nc.sync.dma_start(
                        y[i, :, :, 0:2, :], ot01[:]))
                elif rb == 3:
                    chunks.append(lambda: nc.sync.dma_start(
                        y[i, :, :, 2:4, :], ot23[:]))
            return chunks

        def interleave(a, b):
            """Emit chunk lists a and b proportionally merged."""
            if not a:
                a, b = b, []
            kb = 0
            for ka, fa in enumerate(a):
                fa()
                want = (ka + 1) * len(b) // len(a)
                while kb < want:
                    b[kb]()
                    kb += 1
            while kb < len(b):
                b[kb]()
                kb += 1

        # Every phase interleaves two independent chunk streams so each
        # engine always has ready work while the other stream waits on a
        # PSUM buffer to drain:
        #   [p1(0) || p1(1)], [p2(0) || p1(2)], [p2(1) || p1(3)],
        #   [p2(2) || p2(3)]
        assert bpc == 4
        t2s = {i: {} for i in range(bpc)}
        interleave(p1_chunks(0, t2s[0]), p1_chunks(1, t2s[1]))
        interleave(p2_chunks(0, t2s[0]),
                   [lambda: load_x(2)] + p1_chunks(2, t2s[2]))
        interleave(p2_chunks(1, t2s[1]),
                   [lambda: load_x(3)] + p1_chunks(3, t2s[3]))
        interleave(p2_chunks(2, t2s[2]), p2_chunks(3, t2s[3]))
    return nc


_CACHED = {}


def _get_nc_and_wmat():
    if "nc" not in _CACHED:
        ops = build_ops()
        wmat = build_bands(ops).astype(NPDT)
        # device layout [128, NT, 4*SLOT]
        _CACHED["wmat"] = np.ascontiguousarray(wmat.transpose(1, 0, 2))
        nc = build_nc()
        if not nc.is_finalized():
            nc.finalize()
        _CACHED["nc"] = nc
    return _CACHED["nc"], _CACHED["wmat"]


def prepare_in_maps(x):
    """x: (32, 1, 1024, 1024) float32 -> per-core input dicts."""
    nc, wmat = _get_nc_and_wmat()
    x16 = np.asarray(x)[:, 0].astype(NPDT)
    xw = np.empty((B_TOTAL, 128, NT, H), NPDT)
    for t in range(NT):
        rs = ROW_START[t]
        xw[:, :, t, :] = x16[:, rs:rs + 128, :]
    return nc, [
        {"x": xw[i * BPC:(i + 1) * BPC], "w": wmat}
        for i in range(N_CORES)
    ]


def postprocess(results):
    out = np.concatenate(
        [np.asarray(r["y"]).transpose(0, 2, 3, 1, 4).reshape(BPC, 6, 512, 512)
         for r in results], axis=0)
    return out.astype(np.float32)


def kernel(x):
    """x: (32, 1, 1024, 1024) float32 -> (32, 6, 512, 512) float32."""
    x = np.asarray(x)
    assert x.shape == (B_TOTAL, 1, H, H), x.shape
    nc, in_maps = prepare_in_maps(x)
    res = run_bass_kernel_spmd(nc, in_maps, list(range(N_CORES)))
    return postprocess(res.results)


# revision 14
# speedup vs baseline: 1.2020x; 1.2020x over previous
"""Trainium2 Bass kernel for nn_DWTExtractor.

Computes, for each single-channel 1024x1024 image, 6 output channels
(3 Haar DWT2 details + 3 Coif1 DWT2 details bilinearly resized to 512x512).

Everything is linear and separable, so each channel is
    chan = RowM @ img @ ColM^T
with RowM/ColM in {Hlo, Hhi, RClo, RChi} (all [512, 1024] banded operators;
RC* fold the coif1 DWT with the jax.image.resize 514->512 linear+antialias
matrix). Both passes run on the TensorEngine with the *data* as the
stationary operand (lhsT), so each pass's output lands in PSUM already
transposed for the next pass - no transpose instructions at all:

  pass 1: T2[op][w, n] = sum_h X[h, w] * Op[n, h]
          lhsT = X[h-window, w-window] (128x128), rhs = packed band matrix
  pass 2: chan[m, n]   = sum_w T2[op][w, m] * Col[n, w]
          lhsT = T2[w-window, m-slice], rhs = band slice

The image axes are covered by 9 overlapping 128-wide windows (stride 114)
so that every output column's 12-tap support lies inside a single window;
each window writes a disjoint column slice (singleton PSUM groups, no
cross-window accumulation). Pass-1 packs all 4 operators' bands into one
[128, 228] rhs per window -> one matmul per (w-window, h-window).

v2 over the baseline:
  - output in fp16 (host upcasts): halves the dominant HBM write traffic
  - input pre-windowed on host -> 1 input DMA per image, 1 weight DMA,
    2 output DMAs per image (13 dma_start total vs 141; each trigger
    costs ~630ns serialized on the Sync engine / HWDGE)
  - PSUM evacuation split across Vector+Scalar+GpSimd (3 engines)
  - pass-1 (image i+1) chunks interleaved with pass-2 (image i) chunks
    in the in-order PE queue so PSUM-reuse waits never stall the PE

Sharding: pure data parallel, 32 images -> 8 cores x 4 images.
"""

import sys

sys.path.insert(0, "/opt/trn_rl_repo")

from contextlib import ExitStack

import numpy as np

import concourse.bass as bass
from concourse import bacc
import concourse.mybir as mybir
import concourse.tile as tile
from concourse.bass_utils import run_bass_kernel_spmd

# ---------------------------------------------------------------------------
# Host-side operator construction (pure numpy, float64)
# ---------------------------------------------------------------------------

_c = np.array([-0.01565572813546454, -0.0727326195128539, 0.38486484686420286,
               0.8525720202122554, 0.3378976624578092, -0.0727326195128539])
HAAR_LO = np.array([0.7071067811865476, 0.7071067811865476])
HAAR_HI = np.array([-0.7071067811865476, 0.7071067811865476])
COIF1_LO = _c.copy()
COIF1_HI = ((-1.0) ** (np.arange(6) + 1)) * _c[::-1]

H = 1024
NT = 9            # overlapping 128-row windows, stride 114
SLOT = 57         # output columns assigned per window (57 * 9 = 513)
ROW_START = [min(max(114 * t - 6, 0), H - 128) for t in range(NT)]
N_CORES = 8
B_TOTAL = 32
BPC = B_TOTAL // N_CORES

# channel -> (row op index, col op index); ops are [Hlo, Hhi, RClo, RChi]
CHAN_OPS = [(1, 0), (0, 1), (1, 1), (3, 2), (2, 3), (3, 3)]

DT = mybir.dt.float16
NPDT = np.float16
F32 = mybir.dt.float32


def _dwt1d_np(x, filt):
    L = len(filt)
    n = x.shape[-1]
    xp = np.pad(x, [(0, 0)] * (x.ndim - 1) + [(L - 1, L - 1)], mode="symmetric")
    out_len = (n + L - 1) // 2
    fr = filt[::-1]
    y = np.zeros(x.shape[:-1] + (out_len,), dtype=x.dtype)
    for j in range(L):
        y = y + fr[j] * xp[..., 1 + j:1 + j + 2 * out_len:2]
    return y


def _dwt_matrix(n, filt):
    eye = np.eye(n, dtype=np.float64)
    return _dwt1d_np(eye, np.asarray(filt, np.float64)).T.copy()


def _resize_matrix(in_size, out_size):
    """Replicates jax.image.resize(method='linear', antialias=True)."""
    scale = out_size / in_size
    inv_scale = 1.0 / scale
    kernel_scale = max(inv_scale, 1.0)
    sample_f = (np.arange(out_size, dtype=np.float64) + 0.5) * inv_scale - 0.5
    x = np.abs(sample_f[None, :]
               - np.arange(in_size, dtype=np.float64)[:, None]) / kernel_scale
    w = np.maximum(0.0, 1.0 - x)
    total = w.sum(axis=0, keepdims=True)
    w = np.where(np.abs(total) > 1000.0 * np.finfo(np.float32).eps,
                 w / np.where(total != 0, total, 1), 0.0)
    w = np.where(((sample_f >= -0.5) & (sample_f <= in_size - 0.5))[None, :],
                 w, 0.0)
    return w.T.copy()


def build_ops():
    Hlo = _dwt_matrix(H, HAAR_LO)
    Hhi = _dwt_matrix(H, HAAR_HI)
    Clo = _dwt_matrix(H, COIF1_LO)
    Chi = _dwt_matrix(H, COIF1_HI)
    R = _resize_matrix(514, 512)
    return [Hlo, Hhi, R @ Clo, R @ Chi]


def assigned(t):
    return SLOT * t, min(SLOT * (t + 1), 512)


def build_bands(ops):
    """wmat [NT, 128, 4*SLOT]: per-window packed band matrices."""
    wmat = np.zeros((NT, 128, 4 * SLOT), np.float64)
    for t in range(NT):
        rs = ROW_START[t]
        n0, n1 = assigned(t)
        for f in range(4):
            full = ops[f][n0:n1]
            mask = np.zeros(H, bool)
            mask[rs:rs + 128] = True
            assert np.abs(full[:, ~mask]).max() == 0.0, (t, f)
            wmat[t, :, f * SLOT:f * SLOT + (n1 - n0)] = full[:, rs:rs + 128].T
    return wmat


# ---------------------------------------------------------------------------
# Bass kernel
# ---------------------------------------------------------------------------

def build_nc(bpc=BPC):
    nc = bacc.Bacc("TRN2", num_swdge_queues=4)
    # x pre-windowed on host: x[i, p, t, w] = img[i, ROW_START[t]+p, w]
    x = nc.dram_tensor("x", [bpc, 128, NT, H], DT, kind="ExternalInput")
    w = nc.dram_tensor("w", [128, NT, 4 * SLOT], DT, kind="ExternalInput")
    # y[i, p, c, rb, n] = chan[c][128*rb + p, n]
    y = nc.dram_tensor("y", [bpc, 128, 6, 4, 512], DT, kind="ExternalOutput")

    with tile.TileContext(nc) as tc, ExitStack() as ctx:
        const = ctx.enter_context(tc.tile_pool(name="const", bufs=1))
        xpool = ctx.enter_context(tc.tile_pool(name="xpool", bufs=1))
        t2p = ctx.enter_context(tc.tile_pool(name="t2p", bufs=2))
        outs = ctx.enter_context(tc.tile_pool(name="outs", bufs=2))
        psum = ctx.enter_context(tc.tile_pool(name="psum", bufs=1,
                                              space="PSUM"))

        xtiles = {}

        def load_x(i):
            xt = xpool.tile([128, NT, H], DT, name=f"x{i}", tag=f"x{i % 3}")
            nc.sync.dma_start(xt[:], x[i])
            xtiles[i] = xt

        # weights first (tiny; pass-1 needs them before anything runs),
        # then the first images' big transfers.
        wt = const.tile([128, NT, 4 * SLOT], DT, name="w", tag="w")
        nc.sync.dma_start(wt[:], w[:])
        for i in range(min(2, bpc)):
            load_x(i)

        def eng_copy(k, dst, src):
            # GPSIMD cannot access PSUM; only Vector/Scalar can evacuate.
            if k == 0:
                nc.vector.tensor_copy(dst, src)
            else:
                nc.scalar.copy(dst, src)

        def p1_chunks(i, t2s):
            """9 chunks: per w-window, 9 matmuls + 2-engine evacuation.

            PSUM is split A/B (slots 0-4 / 5-8) so the copies of one half
            overlap the matmuls of the other half; t2 is split by operator
            pair (ops 0,1 -> t2a written only by Vector, ops 2,3 -> t2b
            written only by Scalar) so the two evacuation copies never
            serialize on a shared destination tile.
            """
            def chunk(wt_i):
                ws = ROW_START[wt_i]
                t2a = t2p.tile([128, 2 * 513], DT,
                               name=f"t2a_{wt_i}", tag=f"t2a_{wt_i}")
                t2b = t2p.tile([128, 2 * 513], DT,
                               name=f"t2b_{wt_i}", tag=f"t2b_{wt_i}")
                t2ar = t2a.rearrange("p (f s j) -> p s f j",
                                     f=2, s=NT, j=SLOT)
                t2br = t2b.rearrange("p (f s j) -> p s f j",
                                     f=2, s=NT, j=SLOT)
                for half in range(2):
                    s0, s1 = (0, 5) if half == 0 else (5, NT)
                    pt = psum.tile([128, 256 * (s1 - s0)], F32,
                                   name=f"pt{half}", tag=f"pt{half}")
                    for ht in range(s0, s1):
                        nc.tensor.matmul(
                            pt[:, 256 * (ht - s0):256 * (ht - s0) + 4 * SLOT],
                            lhsT=xtiles[i][:, ht, ws:ws + 128],
                            rhs=wt[:, ht, :],
                            start=True, stop=True)
                    src = pt.rearrange("p (s c) -> p s c", c=256)[
                        :, :, 0:4 * SLOT].rearrange(
                        "p s (f j) -> p s f j", j=SLOT)
                    nc.vector.tensor_copy(t2ar[:, s0:s1], src[:, :, 0:2])
                    nc.scalar.copy(t2br[:, s0:s1], src[:, :, 2:4])
                t2s[wt_i] = (t2a, t2b)
            return [lambda wt_i=wt_i: chunk(wt_i) for wt_i in range(NT)]

        def p2_chunks(i, t2s):
            """16 matmul/copy chunks + 2 output-DMA chunks for image i."""
            ot01 = outs.tile([128, 6, 2, 512], DT, name="o01", tag="o01")
            ot23 = outs.tile([128, 6, 2, 512], DT, name="o23", tag="o23")
            rr = [0]
            chunks = []

            def chunk(rb, group, ot):
                ptc = {}
                for c in group:
                    ptc[c] = psum.tile([128, 512], F32,
                                       name=f"pc{c}", tag="pc", bufs=3)
                for wt_i in range(NT):
                    n0, n1 = assigned(wt_i)
                    for c in group:
                        ri, ci = CHAN_OPS[c]
                        t2t = t2s[wt_i][ri // 2]
                        nc.tensor.matmul(
                            ptc[c][:, n0:n1],
                            lhsT=t2t[:, 513 * (ri % 2) + 128 * rb:
                                     513 * (ri % 2) + 128 * rb + 128],
                            rhs=wt[:, wt_i, SLOT * ci:SLOT * ci + (n1 - n0)],
                            start=True, stop=True)
                for c in group:
                    # 11-of-24 to Vector, 13 to Scalar: with the 1:1 pass-1
                    # split this balances DVE 0.96GHz vs Act 1.2GHz + the
                    # larger per-op access overhead on Act.
                    eng_copy(0 if (rr[0] % 2 == 0 and rr[0] % 24 != 22) else 1,
                             ot[:, c, rb % 2, :], ptc[c][:])
                    rr[0] += 1

            for rb in range(4):
                ot = ot01 if rb < 2 else ot23
                for group in ((0, 2), (1,), (3, 5), (4,)):
                    chunks.append(
                        lambda rb=rb, group=group, ot=ot: chunk(rb, group, ot))
                if rb == 1:
                    chunks.append(lambda: nc.sync.dma_start(
                        y[i, :, :, 0:2, :], ot01[:]))
                elif rb == 3:
                    chunks.append(lambda: nc.sync.dma_start(
                        y[i, :, :, 2:4, :], ot23[:]))
            return chunks

        def interleave(a, b):
            """Emit chunk lists a and b proportionally merged."""
            if not a:
                a, b = b, []
            kb = 0
            for ka, fa in enumerate(a):
                fa()
                want = (ka + 1) * len(b) // len(a)
                while kb < want:
                    b[kb]()
                    kb += 1
            while kb < len(b):
                b[kb]()
                kb += 1

        # Software pipeline at the image level: pass-1 of image i is
        # interleaved chunk-by-chunk with pass-2 of image i-1 so the
        # PE always has independent ready work while a chunk waits for
        # its PSUM buffer to drain.
        t2s = {i: {} for i in range(bpc)}
        pending = None
        for i in range(bpc):
            c1 = p1_chunks(i, t2s[i])
            if i + 2 < bpc:
                c1 = [lambda j=i + 2: load_x(j)] + c1
            interleave(c1, p2_chunks(*pending) if pending else [])
            pending = (i, t2s[i])
        interleave(p2_chunks(*pending), [])
    return nc


_CACHED = {}


def _get_nc_and_wmat():
    if "nc" not in _CACHED:
        ops = build_ops()
        wmat = build_bands(ops).astype(NPDT)
        # device layout [128, NT, 4*SLOT]
        _CACHED["wmat"] = np.ascontiguousarray(wmat.transpose(1, 0, 2))
        nc = build_nc()
        if not nc.is_finalized():
            nc.finalize()
        _CACHED["nc"] = nc
    return _CACHED["nc"], _CACHED["wmat"]


def prepare_in_maps(x):
    """x: (32, 1, 1024, 1024) float32 -> per-core input dicts."""
    nc, wmat = _get_nc_and_wmat()
    x16 = np.asarray(x)[:, 0].astype(NPDT)
    xw = np.empty((B_TOTAL, 128, NT, H), NPDT)
    for t in range(NT):
        rs = ROW_START[t]
        xw[:, :, t, :] = x16[:, rs:rs + 128, :]
    return nc, [
        {"x": xw[i * BPC:(i + 1) * BPC], "w": wmat}
        for i in range(N_CORES)
    ]


def postprocess(results):
    out = np.concatenate(
        [np.asarray(r["y"]).transpose(0, 2, 3, 1, 4).reshape(BPC, 6, 512, 512)
         for r in results], axis=0)
    return out.astype(np.float32)


def kernel(x):
    """x: (32, 1, 1024, 1024) float32 -> (32, 6, 512, 512) float32."""
    x = np.asarray(x)
    assert x.shape == (B_TOTAL, 1, H, H), x.shape
    nc, in_maps = prepare_in_maps(x)
    res = run_bass_kernel_spmd(nc, in_maps, list(range(N_CORES)))
    return postprocess(res.results)


# revision 17
# speedup vs baseline: 1.2068x; 1.0040x over previous
"""Trainium2 Bass kernel for nn_DWTExtractor.

Computes, for each single-channel 1024x1024 image, 6 output channels
(3 Haar DWT2 details + 3 Coif1 DWT2 details bilinearly resized to 512x512).

Everything is linear and separable, so each channel is
    chan = RowM @ img @ ColM^T
with RowM/ColM in {Hlo, Hhi, RClo, RChi} (all [512, 1024] banded operators;
RC* fold the coif1 DWT with the jax.image.resize 514->512 linear+antialias
matrix). Both passes run on the TensorEngine with the *data* as the
stationary operand (lhsT), so each pass's output lands in PSUM already
transposed for the next pass - no transpose instructions at all:

  pass 1: T2[op][w, n] = sum_h X[h, w] * Op[n, h]
          lhsT = X[h-window, w-window] (128x128), rhs = packed band matrix
  pass 2: chan[m, n]   = sum_w T2[op][w, m] * Col[n, w]
          lhsT = T2[w-window, m-slice], rhs = band slice

The image axes are covered by 9 overlapping 128-wide windows (stride 114)
so that every output column's 12-tap support lies inside a single window;
each window writes a disjoint column slice (singleton PSUM groups, no
cross-window accumulation). Pass-1 packs all 4 operators' bands into one
[128, 228] rhs per window -> one matmul per (w-window, h-window).

v2 over the baseline:
  - output in fp16 (host upcasts): halves the dominant HBM write traffic
  - input pre-windowed on host -> 1 input DMA per image, 1 weight DMA,
    2 output DMAs per image (13 dma_start total vs 141; each trigger
    costs ~630ns serialized on the Sync engine / HWDGE)
  - PSUM evacuation split across Vector+Scalar+GpSimd (3 engines)
  - pass-1 (image i+1) chunks interleaved with pass-2 (image i) chunks
    in the in-order PE queue so PSUM-reuse waits never stall the PE

Sharding: pure data parallel, 32 images -> 8 cores x 4 images.
"""

import sys

sys.path.insert(0, "/opt/trn_rl_repo")

from contextlib import ExitStack

import numpy as np

import concourse.bass as bass
from concourse import bacc
import concourse.mybir as mybir
import concourse.tile as tile
from concourse.bass_utils import run_bass_kernel_spmd

# ---------------------------------------------------------------------------
# Host-side operator construction (pure numpy, float64)
# ---------------------------------------------------------------------------

_c = np.array([-0.01565572813546454, -0.0727326195128539, 0.38486484686420286,
               0.8525720202122554, 0.3378976624578092, -0.0727326195128539])
HAAR_LO = np.array([0.7071067811865476, 0.7071067811865476])
HAAR_HI = np.array([-0.7071067811865476, 0.7071067811865476])
COIF1_LO = _c.copy()
COIF1_HI = ((-1.0) ** (np.arange(6) + 1)) * _c[::-1]

H = 1024
NT = 9            # overlapping 128-row windows, stride 114
SLOT = 57         # output columns assigned per window (57 * 9 = 513)
ROW_START = [min(max(114 * t - 6, 0), H - 128) for t in range(NT)]
N_CORES = 8
B_TOTAL = 32
BPC = B_TOTAL // N_CORES

# channel -> (row op index, col op index); ops are [Hlo, Hhi, RClo, RChi]
CHAN_OPS = [(1, 0), (0, 1), (1, 1), (3, 2), (2, 3), (3, 3)]

DT = mybir.dt.float16
NPDT = np.float16
F32 = mybir.dt.float32


def _dwt1d_np(x, filt):
    L = len(filt)
    n = x.shape[-1]
    xp = np.pad(x, [(0, 0)] * (x.ndim - 1) + [(L - 1, L - 1)], mode="symmetric")
    out_len = (n + L - 1) // 2
    fr = filt[::-1]
    y = np.zeros(x.shape[:-1] + (out_len,), dtype=x.dtype)
    for j in range(L):
        y = y + fr[j] * xp[..., 1 + j:1 + j + 2 * out_len:2]
    return y


def _dwt_matrix(n, filt):
    eye = np.eye(n, dtype=np.float64)
    return _dwt1d_np(eye, np.asarray(filt, np.float64)).T.copy()


def _resize_matrix(in_size, out_size):
    """Replicates jax.image.resize(method='linear', antialias=True)."""
    scale = out_size / in_size
    inv_scale = 1.0 / scale
    kernel_scale = max(inv_scale, 1.0)
    sample_f = (np.arange(out_size, dtype=np.float64) + 0.5) * inv_scale - 0.5
    x = np.abs(sample_f[None, :]
               - np.arange(in_size, dtype=np.float64)[:, None]) / kernel_scale
    w = np.maximum(0.0, 1.0 - x)
    total = w.sum(axis=0, keepdims=True)
    w = np.where(np.abs(total) > 1000.0 * np.finfo(np.float32).eps,
                 w / np.where(total != 0, total, 1), 0.0)
    w = np.where(((sample_f >= -0.5) & (sample_f <= in_size - 0.5))[None, :],
                 w, 0.0)
    return w.T.copy()


def build_ops():
    Hlo = _dwt_matrix(H, HAAR_LO)
    Hhi = _dwt_matrix(H, HAAR_HI)
    Clo = _dwt_matrix(H, COIF1_LO)
    Chi = _dwt_matrix(H, COIF1_HI)
    R = _resize_matrix(514, 512)
    return [Hlo, Hhi, R @ Clo, R @ Chi]


def assigned(t):
    return SLOT * t, min(SLOT * (t + 1), 512)


def build_bands(ops):
    """wmat [NT, 128, 4*SLOT]: per-window packed band matrices."""
    wmat = np.zeros((NT, 128, 4 * SLOT), np.float64)
    for t in range(NT):
        rs = ROW_START[t]
        n0, n1 = assigned(t)
        for f in range(4):
            full = ops[f][n0:n1]
            mask = np.zeros(H, bool)
            mask[rs:rs + 128] = True
            assert np.abs(full[:, ~mask]).max() == 0.0, (t, f)
            wmat[t, :, f * SLOT:f * SLOT + (n1 - n0)] = full[:, rs:rs + 128].T
    return wmat


# ---------------------------------------------------------------------------
# Bass kernel
# ---------------------------------------------------------------------------

def build_nc(bpc=BPC):
    nc = bacc.Bacc("TRN2", num_swdge_queues=4)
    # x pre-windowed on host: x[i, p, t, w] = img[i, ROW_START[t]+p, w]
    x = nc.dram_tensor("x", [bpc, 128, NT, H], DT, kind="ExternalInput")
    w = nc.dram_tensor("w", [128, NT, 4 * SLOT], DT, kind="ExternalInput")
    # y[i, p, c, rb, n] = chan[c][128*rb + p, n]
    y = nc.dram_tensor("y", [bpc, 128, 6, 4, 512], DT, kind="ExternalOutput")

    with tile.TileContext(nc) as tc, ExitStack() as ctx:
        const = ctx.enter_context(tc.tile_pool(name="const", bufs=1))
        xpool = ctx.enter_context(tc.tile_pool(name="xpool", bufs=1))
        t2p = ctx.enter_context(tc.tile_pool(name="t2p", bufs=2))
        outs = ctx.enter_context(tc.tile_pool(name="outs", bufs=2))
        psum = ctx.enter_context(tc.tile_pool(name="psum", bufs=1,
                                              space="PSUM"))

        xtiles = {}
        XSPLIT = 640   # w-windows 0-4 live in [0,640), 5-8 in [564,1024)
        XBOFF = 564

        def load_x(i):
            # column-split so pass-1 w-windows 0-4 can start before the
            # right half of the image has arrived
            xa = xpool.tile([128, NT, XSPLIT], DT,
                            name=f"xa{i}", tag=f"xa{i % 2}")
            xb = xpool.tile([128, NT, H - XBOFF], DT,
                            name=f"xb{i}", tag=f"xb{i % 2}")
            nc.sync.dma_start(xa[:], x[i, :, :, 0:XSPLIT])
            nc.sync.dma_start(xb[:], x[i, :, :, XBOFF:H])
            xtiles[i] = (xa, xb)

        # weights first (tiny; pass-1 needs them before anything runs),
        # then the first images' big transfers.
        wt = const.tile([128, NT, 4 * SLOT], DT, name="w", tag="w")
        nc.sync.dma_start(wt[:], w[:])
        for i in range(min(2, bpc)):
            load_x(i)

        def eng_copy(k, dst, src):
            # GPSIMD cannot access PSUM; only Vector/Scalar can evacuate.
            if k == 0:
                nc.vector.tensor_copy(dst, src)
            else:
                nc.scalar.copy(dst, src)

        def p1_chunks(i, t2s):
            """9 chunks: per w-window, 9 matmuls + 2-engine evacuation.

            PSUM is split A/B (slots 0-4 / 5-8) so the copies of one half
            overlap the matmuls of the other half; t2 is split by operator
            pair (ops 0,1 -> t2a written only by Vector, ops 2,3 -> t2b
            written only by Scalar) so the two evacuation copies never
            serialize on a shared destination tile.
            """
            def chunk(wt_i):
                ws = ROW_START[wt_i]
                if ws + 128 <= XSPLIT:
                    xt, xoff = xtiles[i][0], ws
                else:
                    xt, xoff = xtiles[i][1], ws - XBOFF
                t2a = t2p.tile([128, 2 * 513], DT,
                               name=f"t2a_{wt_i}", tag=f"t2a_{wt_i}")
                t2b = t2p.tile([128, 2 * 513], DT,
                               name=f"t2b_{wt_i}", tag=f"t2b_{wt_i}")
                t2ar = t2a.rearrange("p (f s j) -> p s f j",
                                     f=2, s=NT, j=SLOT)
                t2br = t2b.rearrange("p (f s j) -> p s f j",
                                     f=2, s=NT, j=SLOT)
                for half in range(2):
                    s0, s1 = (0, 5) if half == 0 else (5, NT)
                    pt = psum.tile([128, 256 * (s1 - s0)], F32,
                                   name=f"pt{half}", tag=f"pt{half}")
                    for ht in range(s0, s1):
                        nc.tensor.matmul(
                            pt[:, 256 * (ht - s0):256 * (ht - s0) + 4 * SLOT],
                            lhsT=xt[:, ht, xoff:xoff + 128],
                            rhs=wt[:, ht, :],
                            start=True, stop=True)
                    src = pt.rearrange("p (s c) -> p s c", c=256)[
                        :, :, 0:4 * SLOT].rearrange(
                        "p s (f j) -> p s f j", j=SLOT)
                    nc.vector.tensor_copy(t2ar[:, s0:s1], src[:, :, 0:2])
                    nc.scalar.copy(t2br[:, s0:s1], src[:, :, 2:4])
                t2s[wt_i] = (t2a, t2b)
            return [lambda wt_i=wt_i: chunk(wt_i) for wt_i in range(NT)]

        def p2_chunks(i, t2s):
            """16 matmul/copy chunks + 4 output-DMA chunks for image i."""
            ots = [outs.tile([128, 6, 512], DT, name=f"o{rb}", tag=f"o{rb}")
                   for rb in range(4)]
            rr = [0]
            chunks = []

            def chunk(rb, group):
                ptc = {}
                for c in group:
                    ptc[c] = psum.tile([128, 512], F32,
                                       name=f"pc{c}", tag="pc", bufs=3)
                for wt_i in range(NT):
                    n0, n1 = assigned(wt_i)
                    for c in group:
                        ri, ci = CHAN_OPS[c]
                        t2t = t2s[wt_i][ri // 2]
                        nc.tensor.matmul(
                            ptc[c][:, n0:n1],
                            lhsT=t2t[:, 513 * (ri % 2) + 128 * rb:
                                     513 * (ri % 2) + 128 * rb + 128],
                            rhs=wt[:, wt_i, SLOT * ci:SLOT * ci + (n1 - n0)],
                            start=True, stop=True)
                for c in group:
                    # 11-of-24 to Vector, 13 to Scalar: with the 1:1 pass-1
                    # split this balances DVE 0.96GHz vs Act 1.2GHz + the
                    # larger per-op access overhead on Act.
                    eng_copy(0 if (rr[0] % 2 == 0 and rr[0] % 24 != 22) else 1,
                             ots[rb][:, c, :], ptc[c][:])
                    rr[0] += 1

            for rb in range(4):
                for group in ((0, 2), (1,), (3, 5), (4,)):
                    chunks.append(
                        lambda rb=rb, group=group: chunk(rb, group))
                chunks.append(lambda rb=rb: nc.sync.dma_start(
                    y[i, :, :, rb, :], ots[rb][:]))
            return chunks

        def interleave(a, b):
            """Emit chunk lists a and b proportionally merged."""
            if not a:
                a, b = b, []
            kb = 0
            for ka, fa in enumerate(a):
                fa()
                want = (ka + 1) * len(b) // len(a)
                while kb < want:
                    b[kb]()
                    kb += 1
            while kb < len(b):
                b[kb]()
                kb += 1

        # Software pipeline at the image level: pass-1 of image i is
        # interleaved chunk-by-chunk with pass-2 of image i-1 so the
        # PE always has independent ready work while a chunk waits for
        # its PSUM buffer to drain.
        t2s = {i: {} for i in range(bpc)}
        pending = None
        for i in range(bpc):
            c1 = p1_chunks(i, t2s[i])
            if i + 2 < bpc:
                c1 = [lambda j=i + 2: load_x(j)] + c1
            interleave(c1, p2_chunks(*pending) if pending else [])
            pending = (i, t2s[i])
        interleave(p2_chunks(*pending), [])
    return nc


_CACHED = {}


def _get_nc_and_wmat():
    if "nc" not in _CACHED:
        ops = build_ops()
        wmat = build_bands(ops).astype(NPDT)
        # device layout [128, NT, 4*SLOT]
        _CACHED["wmat"] = np.ascontiguousarray(wmat.transpose(1, 0, 2))
        nc = build_nc()
        if not nc.is_finalized():
            nc.finalize()
        _CACHED["nc"] = nc
    return _CACHED["nc"], _CACHED["wmat"]


def prepare_in_maps(x):
    """x: (32, 1, 1024, 1024) float32 -> per-core input dicts."""
    nc, wmat = _get_nc_and_wmat()
    x16 = np.asarray(x)[:, 0].astype(NPDT)
    xw = np.empty((B_TOTAL, 128, NT, H), NPDT)
    for t in range(NT):
        rs = ROW_START[t]
        xw[:, :, t, :] = x16[:, rs:rs + 128, :]
    return nc, [
        {"x": xw[i * BPC:(i + 1) * BPC], "w": wmat}
        for i in range(N_CORES)
    ]


def postprocess(results):
    out = np.concatenate(
        [np.asarray(r["y"]).transpose(0, 2, 3, 1, 4).reshape(BPC, 6, 512, 512)
         for r in results], axis=0)
    return out.astype(np.float32)


def kernel(x):
    """x: (32, 1, 1024, 1024) float32 -> (32, 6, 512, 512) float32."""
    x = np.asarray(x)
    assert x.shape == (B_TOTAL, 1, H, H), x.shape
    nc, in_maps = prepare_in_maps(x)
    res = run_bass_kernel_spmd(nc, in_maps, list(range(N_CORES)))
    return postprocess(res.results)


# revision 18
# speedup vs baseline: 1.2762x; 1.0575x over previous
"""Trainium2 Bass kernel for nn_DWTExtractor.

Computes, for each single-channel 1024x1024 image, 6 output channels
(3 Haar DWT2 details + 3 Coif1 DWT2 details bilinearly resized to 512x512).

Everything is linear and separable, so each channel is
    chan = RowM @ img @ ColM^T
with RowM/ColM in {Hlo, Hhi, RClo, RChi} (all [512, 1024] banded operators;
RC* fold the coif1 DWT with the jax.image.resize 514->512 linear+antialias
matrix). Both passes run on the TensorEngine with the *data* as the
stationary operand (lhsT), so each pass's output lands in PSUM already
transposed for the next pass - no transpose instructions at all:

  pass 1: T2[op][w, n] = sum_h X[h, w] * Op[n, h]
          lhsT = X[h-window, w-window] (128x128), rhs = packed band matrix
  pass 2: chan[m, n]   = sum_w T2[op][w, m] * Col[n, w]
          lhsT = T2[w-window, m-slice], rhs = band slice

The image axes are covered by 9 overlapping 128-wide windows (stride 114)
so that every output column's 12-tap support lies inside a single window;
each window writes a disjoint column slice (singleton PSUM groups, no
cross-window accumulation). Pass-1 packs all 4 operators' bands into one
[128, 228] rhs per window -> one matmul per (w-window, h-window).

The output is written in fp16 (the host upcasts to fp32): output bytes
are 2/3 of all HBM traffic at fp32, and the tolerance budget allows it.

Sharding: pure data parallel, 32 images -> 8 cores x 4 images.
"""

import sys

sys.path.insert(0, "/opt/trn_rl_repo")

from contextlib import ExitStack

import numpy as np

import concourse.bass as bass
from concourse import bacc
import concourse.mybir as mybir
import concourse.tile as tile
from concourse.bass_utils import run_bass_kernel_spmd

# ---------------------------------------------------------------------------
# Host-side operator construction (pure numpy, float64)
# ---------------------------------------------------------------------------

_c = np.array([-0.01565572813546454, -0.0727326195128539, 0.38486484686420286,
               0.8525720202122554, 0.3378976624578092, -0.0727326195128539])
HAAR_LO = np.array([0.7071067811865476, 0.7071067811865476])
HAAR_HI = np.array([-0.7071067811865476, 0.7071067811865476])
COIF1_LO = _c.copy()
COIF1_HI = ((-1.0) ** (np.arange(6) + 1)) * _c[::-1]

H = 1024
NT = 9            # overlapping 128-row windows, stride 114
SLOT = 57         # output columns assigned per window (57 * 9 = 513)
ROW_START = [min(max(114 * t - 6, 0), H - 128) for t in range(NT)]
N_CORES = 8
B_TOTAL = 32
BPC = B_TOTAL // N_CORES

# channel -> (row op index, col op index); ops are [Hlo, Hhi, RClo, RChi]
CHAN_OPS = [(1, 0), (0, 1), (1, 1), (3, 2), (2, 3), (3, 3)]
# channel emission order: channels sharing the same pass-1 tensor adjacent
CH_ORDER = [0, 2, 1, 3, 5, 4]

DT = mybir.dt.float16
NPDT = np.float16


def _dwt1d_np(x, filt):
    L = len(filt)
    n = x.shape[-1]
    xp = np.pad(x, [(0, 0)] * (x.ndim - 1) + [(L - 1, L - 1)], mode="symmetric")
    out_len = (n + L - 1) // 2
    fr = filt[::-1]
    y = np.zeros(x.shape[:-1] + (out_len,), dtype=x.dtype)
    for j in range(L):
        y = y + fr[j] * xp[..., 1 + j:1 + j + 2 * out_len:2]
    return y


def _dwt_matrix(n, filt):
    eye = np.eye(n, dtype=np.float64)
    return _dwt1d_np(eye, np.asarray(filt, np.float64)).T.copy()


def _resize_matrix(in_size, out_size):
    """Replicates jax.image.resize(method='linear', antialias=True)."""
    scale = out_size / in_size
    inv_scale = 1.0 / scale
    kernel_scale = max(inv_scale, 1.0)
    sample_f = (np.arange(out_size, dtype=np.float64) + 0.5) * inv_scale - 0.5
    x = np.abs(sample_f[None, :]
               - np.arange(in_size, dtype=np.float64)[:, None]) / kernel_scale
    w = np.maximum(0.0, 1.0 - x)
    total = w.sum(axis=0, keepdims=True)
    w = np.where(np.abs(total) > 1000.0 * np.finfo(np.float32).eps,
                 w / np.where(total != 0, total, 1), 0.0)
    w = np.where(((sample_f >= -0.5) & (sample_f <= in_size - 0.5))[None, :],
                 w, 0.0)
    return w.T.copy()


def build_ops():
    Hlo = _dwt_matrix(H, HAAR_LO)
    Hhi = _dwt_matrix(H, HAAR_HI)
    Clo = _dwt_matrix(H, COIF1_LO)
    Chi = _dwt_matrix(H, COIF1_HI)
    R = _resize_matrix(514, 512)
    return [Hlo, Hhi, R @ Clo, R @ Chi]


def assigned(t):
    return SLOT * t, min(SLOT * (t + 1), 512)


def build_bands(ops):
    """wmat [NT, 128, 4*SLOT]: per-window packed band matrices."""
    wmat = np.zeros((NT, 128, 4 * SLOT), np.float64)
    for t in range(NT):
        rs = ROW_START[t]
        n0, n1 = assigned(t)
        for f in range(4):
            full = ops[f][n0:n1]
            mask = np.zeros(H, bool)
            mask[rs:rs + 128] = True
            assert np.abs(full[:, ~mask]).max() == 0.0, (t, f)
            wmat[t, :, f * SLOT:f * SLOT + (n1 - n0)] = full[:, rs:rs + 128].T
    return wmat


# ---------------------------------------------------------------------------
# Bass kernel
# ---------------------------------------------------------------------------

def build_nc(bpc=BPC):
    nc = bacc.Bacc("TRN2", num_swdge_queues=4)
    x = nc.dram_tensor("x", [bpc, H, H], DT, kind="ExternalInput")
    w = nc.dram_tensor("w", [NT, 128, 4 * SLOT], DT, kind="ExternalInput")
    y = nc.dram_tensor("y", [bpc, 6, 512, 512], DT, kind="ExternalOutput")

    with tile.TileContext(nc) as tc, ExitStack() as ctx:
        const = ctx.enter_context(tc.tile_pool(name="const", bufs=1))
        xhalf = ctx.enter_context(tc.tile_pool(name="xhalf", bufs=4))
        t2p = ctx.enter_context(tc.tile_pool(name="t2p", bufs=2))
        outs = ctx.enter_context(tc.tile_pool(name="outs", bufs=16))
        psum = ctx.enter_context(tc.tile_pool(name="psum", bufs=1,
                                              space="PSUM"))

        # first image's loads go first so the big transfers start
        # immediately; the small const loads slot in behind them.
        x0_tiles = []
        for t in range(NT):
            xb_t = xhalf.tile([128, H], DT, name=f"xb{t}", tag=f"xb{t}")
            nc.sync.dma_start(
                xb_t[:], x[0, ROW_START[t]:ROW_START[t] + 128, :])
            x0_tiles.append(xb_t)

        wt_tiles = []
        for t in range(NT):
            wt_t = const.tile([128, 4 * SLOT], DT, name=f"w{t}", tag=f"w{t}")
            nc.sync.dma_start(wt_t[:], w[t])
            wt_tiles.append(wt_t)

        def emit_load_pass1(i):
            # ---- load (input pre-cast to fp16 on host) ----
            if i == 0:
                xtiles = x0_tiles
            else:
                xtiles = []
                for t in range(NT):
                    xb_t = xhalf.tile([128, H], DT,
                                      name=f"xb{t}", tag=f"xb{t}")
                    nc.sync.dma_start(
                        xb_t[:], x[i, ROW_START[t]:ROW_START[t] + 128, :])
                    xtiles.append(xb_t)

            # ---- pass 1 ----
            # psum slot layout: slot s (of 9) at col 256*s, 228 used cols
            # (4 ops x 57). ptA holds slots 0-4, ptB slots 5-8.
            t2 = {}
            for wt in range(NT):
                ws = ROW_START[wt]
                ptA = psum.tile([128, 1280], mybir.dt.float32,
                                name="ptA", tag="ptA")
                ptB = psum.tile([128, 1024], mybir.dt.float32,
                                name="ptB", tag="ptB")
                for ht in range(NT):
                    pt, s = (ptA, ht) if ht < 5 else (ptB, ht - 5)
                    nc.tensor.matmul(
                        pt[:, 256 * s:256 * s + 4 * SLOT],
                        lhsT=xtiles[ht][:, ws:ws + 128],
                        rhs=wt_tiles[ht][:],
                        start=True, stop=True)
                # T2 layout is op-major: op f occupies cols [513f, 513f+513),
                # so pass-2 lhsT slices are single-free-dim. The copies
                # de-interleave the psum slot layout via 3-free-dim APs.
                t2t = t2p.tile([128, 4 * 513], DT,
                               name=f"t2_{wt}", tag=f"t2_{wt}")
                t2r = t2t.rearrange("p (f s j) -> p s f j", f=4, s=NT, j=SLOT)
                srcA = ptA.rearrange("p (s c) -> p s c", c=256)[
                    :, :, 0:228].rearrange("p s (f j) -> p s f j", j=SLOT)
                srcB = ptB.rearrange("p (s c) -> p s c", c=256)[
                    :, :, 0:228].rearrange("p s (f j) -> p s f j", j=SLOT)
                if wt % 2 == 0:
                    nc.vector.tensor_copy(t2r[:, 0:5], srcA)
                    nc.scalar.copy(t2r[:, 5:NT], srcB)
                else:
                    nc.scalar.copy(t2r[:, 0:5], srcA)
                    nc.vector.tensor_copy(t2r[:, 5:NT], srcB)
                t2[wt] = t2t
            return t2

        def emit_pass2(i, t2):
            for rb in range(4):
                for group in ((0, 2), (1,), (3, 5), (4,)):
                    ptc = {}
                    for c in group:
                        ptc[c] = psum.tile([128, 512], mybir.dt.float32,
                                           name=f"pc{c}", tag="pc", bufs=3)
                    for wt in range(NT):
                        n0, n1 = assigned(wt)
                        for c in group:
                            ri, ci = CHAN_OPS[c]
                            nc.tensor.matmul(
                                ptc[c][:, n0:n1],
                                lhsT=t2[wt][:, 513 * ri + 128 * rb:
                                            513 * ri + 128 * rb + 128],
                                rhs=wt_tiles[wt][:, SLOT * ci:SLOT * ci + (n1 - n0)],
                                start=True, stop=True)
                    for k, c in enumerate(group):
                        ot = outs.tile([128, 512], DT, name="ot", tag="ot")
                        if (rb + k) % 2 == 0:
                            nc.vector.tensor_copy(ot[:], ptc[c][:])
                        else:
                            nc.scalar.copy(ot[:], ptc[c][:])
                        nc.sync.dma_start(
                            y[i, c, 128 * rb:128 * rb + 128, :], ot[:])

        # Software pipeline at the image level: pass-1 of image i+1 is
        # emitted before pass-2 of image i, so the in-order PE queue has
        # ready work while pass-2 waits on image i's last T2 copies.
        pending = None
        for i in range(bpc):
            t2 = emit_load_pass1(i)
            if pending is not None:
                emit_pass2(*pending)
            pending = (i, t2)
        emit_pass2(*pending)
    return nc


_CACHED = {}


def _get_nc_and_wmat():
    if "nc" not in _CACHED:
        ops = build_ops()
        wmat = build_bands(ops).astype(NPDT)
        _CACHED["wmat"] = wmat
        nc = build_nc()
        if not nc.is_finalized():
            nc.finalize()
        _CACHED["nc"] = nc
    return _CACHED["nc"], _CACHED["wmat"]


def prepare_in_maps(x):
    """x: (32, 1, 1024, 1024) float32 -> per-core input dicts."""
    nc, wmat = _get_nc_and_wmat()
    x = np.ascontiguousarray(np.asarray(x))
    return nc, [
        {"x": np.ascontiguousarray(x[i * BPC:(i + 1) * BPC, 0]).astype(NPDT),
         "w": wmat}
        for i in range(N_CORES)
    ]


def postprocess(results):
    out = np.concatenate([np.asarray(r["y"]) for r in results], axis=0)
    return out.astype(np.float32)


def kernel(x):
    """x: (32, 1, 1024, 1024) float32 -> (32, 6, 512, 512) float32."""
    x = np.asarray(x)
    assert x.shape == (B_TOTAL, 1, H, H), x.shape
    nc, in_maps = prepare_in_maps(x)
    res = run_bass_kernel_spmd(nc, in_maps, list(range(N_CORES)))
    return postprocess(res.results)
